# revision 10
# baseline (speedup 1.0000x reference)
"""Trainium2 Bass kernel for segment-mean pooling + 3-layer conv1x1/BN head.

Full inputs in, full outputs out. Internally shards the batch dim (B=8)
across 8 NeuronCores: each core segment-mean-pools its batch's
features [C=128, N=131072] into pooled [C, P=128] via one-hot matmuls on
the PE, then an AllGather shares pooled across cores and every core
computes the (tiny) MLP head on the full [C, B*P] tensor.
"""

import numpy as np

B, C, N, P, D = 8, 128, 131072, 128, 128
EPS = 1e-5
NCHUNK = N // 128          # 1024 chunks of 128 points
GROUP_PTS = 2048           # points per feature DMA
NGROUP = N // GROUP_PTS    # 64
HALF_PTS = 1024            # points per PSUM transpose tile (8 chunks)

# column offsets in the merged constant tensor [128, CST_W]
O_IOTA = 0
O_LABT = 128
O_IDENT = O_LABT + NCHUNK          # 1152
O_INVC = O_IDENT + 128             # 1280
O_W1T = O_INVC + 128               # 1408
O_W2T = O_W1T + 128                # 1536
O_W3T = O_W2T + 128                # 1664
O_GB = O_W3T + 128                 # 1792
CST_W = O_GB + 6                   # 1798

_compiled = None


def _build():
    from concourse import bacc, mybir, tile
    from contextlib import ExitStack

    f32 = mybir.dt.float32
    bf16 = mybir.dt.bfloat16

    nc = bacc.Bacc("TRN2", target_bir_lowering=False, debug=False, num_devices=8)

    # Per-core inputs (core i gets batch i's slices; constants replicated).
    feat = nc.dram_tensor("feat", [C, N], f32, kind="ExternalInput").ap()
    cst = nc.dram_tensor("cst", [128, CST_W], f32, kind="ExternalInput").ap()
    xout = nc.dram_tensor("xout", [B, D, P], f32, kind="ExternalOutput").ap()

    with tile.TileContext(nc) as tc, ExitStack() as ctx:
        const = ctx.enter_context(tc.tile_pool(name="const", bufs=1))
        dram = ctx.enter_context(tc.tile_pool(name="dram", bufs=1, space="DRAM"))
        cst_sb = const.tile([128, CST_W], f32)
        nc.sync.dma_start(out=cst_sb[:], in_=cst)
        iota_sb = cst_sb[:, O_IOTA : O_IOTA + 128]
        labT_sb = cst_sb[:, O_LABT : O_LABT + NCHUNK]
        ident_sb = cst_sb[:, O_IDENT : O_IDENT + 128]
        invc_sb = cst_sb[:, O_INVC : O_INVC + 128]
        w1t_sb = cst_sb[:, O_W1T : O_W1T + 128]
        w2t_sb = cst_sb[:, O_W2T : O_W2T + 128]
        w3t_sb = cst_sb[:, O_W3T : O_W3T + 128]
        gb_sb = cst_sb[:, O_GB : O_GB + 6]

        # ---------------- Phase 1: segment sums S[c, p] ----------------
        S_sb = const.tile([C, P], f32)
        with (
            tc.tile_pool(name="featg", bufs=3) as featg_pool,
            tc.tile_pool(name="ftp", bufs=2, space="PSUM") as ftp_pool,
            tc.tile_pool(name="sps", bufs=1, space="PSUM") as sps_pool,
            tc.tile_pool(name="ftb", bufs=3) as ftb_pool,
            tc.tile_pool(name="oh", bufs=3) as oh_pool,
        ):
            S_ps = sps_pool.tile([C, P], f32)
            prev = None  # (ftb tile, base chunk index) pending matmuls

            def emit_mms(pend):
                ftb_t, base = pend
                oh8 = oh_pool.tile([128, 8, 128], bf16, tag="oh", name=f"oh{base}")
                nc.vector.tensor_tensor(
                    out=oh8[:],
                    in0=labT_sb[:, base : base + 8].to_broadcast([128, 8, 128]),
                    in1=iota_sb[:, None, :].to_broadcast([128, 8, 128]),
                    op=mybir.AluOpType.is_equal,
                )
                for k in range(8):
                    t = base + k
                    nc.tensor.matmul(
                        out=S_ps[:],
                        lhsT=ftb_t[:, k * 128 : (k + 1) * 128],
                        rhs=oh8[:, k, :],
                        start=(t == 0),
                        stop=(t == NCHUNK - 1),
                    )

            for g in range(NGROUP):
                fg = featg_pool.tile([C, GROUP_PTS], f32, tag="fg", name=f"fg{g}")
                nc.sync.dma_start(
                    out=fg[:], in_=feat[:, g * GROUP_PTS : (g + 1) * GROUP_PTS]
                )
                for h in range(2):
                    ftp = ftp_pool.tile(
                        [128, HALF_PTS], f32, tag="ftp", name=f"ftp{g}_{h}"
                    )
                    for k in range(8):
                        c0 = (h * 8 + k) * 128
                        nc.tensor.transpose(
                            out=ftp[:, k * 128 : (k + 1) * 128],
                            in_=fg[:, c0 : c0 + 128],
                            identity=ident_sb,
                        )
                    ftb = ftb_pool.tile(
                        [128, HALF_PTS], bf16, tag="ftb", name=f"ftb{g}_{h}"
                    )
                    nc.scalar.activation(
                        out=ftb[:], in_=ftp[:], func=mybir.ActivationFunctionType.Copy
                    )
                    if prev is not None:
                        emit_mms(prev)
                    prev = (ftb, (g * 2 + h) * 8)
            emit_mms(prev)

            # pooled = S * inv_counts (inv_counts pre-broadcast on host)
            nc.vector.tensor_tensor(
                out=S_sb[:], in0=S_ps[:], in1=invc_sb, op=mybir.AluOpType.mult
            )

        # ---------------- Phase 2: AllGather + MLP head ----------------
        cc_in = dram.tile([C, P], f32)
        cc_out = dram.tile([B, C, P], f32, addr_space="Shared")
        nc.sync.dma_start(out=cc_in[:], in_=S_sb[:])
        nc.gpsimd.collective_compute(
            "AllGather",
            mybir.AluOpType.bypass,
            replica_groups=[list(range(8))],
            ins=[cc_in.opt()],
            outs=[cc_out.opt()],
        )

        F = B * P  # 1024
        with (
            tc.tile_pool(name="mlp", bufs=1) as mlp,
            tc.tile_pool(name="yps", bufs=2, space="PSUM") as yps_pool,
            tc.tile_pool(name="stat", bufs=1) as stat,
        ):
            x0 = mlp.tile([C, F], f32)
            nc.sync.dma_start(
                out=x0[:].rearrange("c (b p) -> c b p", b=B),
                in_=cc_out[:].rearrange("b c p -> c b p"),
            )

            def bn_layer(x_in, wT_ap, gcol, bcol, relu, out_sb):
                yps = yps_pool.tile([128, F], f32, tag="yps", name=f"yps{gcol}")
                nc.tensor.matmul(
                    out=yps[:, :512], lhsT=wT_ap, rhs=x_in[:, :512],
                    start=True, stop=True,
                )
                nc.tensor.matmul(
                    out=yps[:, 512:], lhsT=wT_ap, rhs=x_in[:, 512:],
                    start=True, stop=True,
                )
                s1 = stat.tile([128, 1], f32, tag=f"s1{gcol}", name=f"s1{gcol}")
                s2 = stat.tile([128, 1], f32, tag=f"s2{gcol}", name=f"s2{gcol}")
                sqv = stat.tile([128, F], f32, tag="sqv", name=f"sqv{gcol}")
                nc.vector.tensor_reduce(
                    out=s1[:], in_=yps[:], axis=mybir.AxisListType.X,
                    op=mybir.AluOpType.add,
                )
                nc.scalar.activation(
                    out=sqv[:], in_=yps[:],
                    func=mybir.ActivationFunctionType.Square, accum_out=s2[:],
                )
                m = stat.tile([128, 1], f32, tag=f"m{gcol}", name=f"m{gcol}")
                v = stat.tile([128, 1], f32, tag=f"v{gcol}", name=f"v{gcol}")
                nc.vector.tensor_scalar(
                    out=m[:], in0=s1[:], scalar1=1.0 / F, scalar2=None,
                    op0=mybir.AluOpType.mult,
                )
                # v = s2/F + eps - m^2
                msq = stat.tile([128, 1], f32, tag=f"msq{gcol}", name=f"msq{gcol}")
                nc.vector.tensor_tensor(
                    out=msq[:], in0=m[:], in1=m[:], op=mybir.AluOpType.mult
                )
                nc.vector.tensor_scalar(
                    out=v[:], in0=s2[:], scalar1=1.0 / F, scalar2=EPS,
                    op0=mybir.AluOpType.mult, op1=mybir.AluOpType.add,
                )
                nc.vector.tensor_tensor(
                    out=v[:], in0=v[:], in1=msq[:], op=mybir.AluOpType.subtract
                )
                std = stat.tile([128, 1], f32, tag=f"std{gcol}", name=f"std{gcol}")
                nc.scalar.activation(
                    out=std[:], in_=v[:], func=mybir.ActivationFunctionType.Sqrt,
                )
                istd = stat.tile([128, 1], f32, tag=f"istd{gcol}", name=f"istd{gcol}")
                nc.vector.reciprocal(out=istd[:], in_=std[:])
                scale = stat.tile(
                    [128, 1], f32, tag=f"scale{gcol}", name=f"scale{gcol}"
                )
                shift = stat.tile(
                    [128, 1], f32, tag=f"shift{gcol}", name=f"shift{gcol}"
                )
                nc.vector.tensor_tensor(
                    out=scale[:], in0=gb_sb[:, gcol : gcol + 1], in1=istd[:],
                    op=mybir.AluOpType.mult,
                )
                # shift = beta - m * scale
                nc.vector.tensor_tensor(
                    out=shift[:], in0=m[:], in1=scale[:], op=mybir.AluOpType.mult
                )
                nc.vector.tensor_tensor(
                    out=shift[:], in0=gb_sb[:, bcol : bcol + 1], in1=shift[:],
                    op=mybir.AluOpType.subtract,
                )
                nc.scalar.activation(
                    out=out_sb[:], in_=yps[:],
                    func=(
                        mybir.ActivationFunctionType.Relu
                        if relu
                        else mybir.ActivationFunctionType.Identity
                    ),
                    scale=scale[:], bias=shift[:],
                )

            x1 = mlp.tile([128, F], f32)
            x2 = mlp.tile([128, F], f32)
            x3 = mlp.tile([128, F], f32)
            bn_layer(x0[:], w1t_sb, 0, 1, True, x1)
            bn_layer(x1[:], w2t_sb, 2, 3, True, x2)
            bn_layer(x2[:], w3t_sb, 4, 5, False, x3)

            nc.sync.dma_start(
                out=xout.rearrange("b d p -> d b p"),
                in_=x3[:].rearrange("d (b p) -> d b p", b=B),
            )

    nc.compile()
    return nc


def _get_compiled():
    global _compiled
    if _compiled is None:
        _compiled = _build()
    return _compiled


def _host_prep(features, labels, W1, g1, beta1, W2, g2, beta2, W3, g3, beta3):
    features = np.asarray(features, dtype=np.float32)
    labels_i = np.asarray(labels).astype(np.int64)

    # host-side: per-batch histogram (counts output + 1/count for the mean)
    offs = labels_i + (np.arange(B, dtype=np.int64)[:, None] * P)
    counts = np.bincount(offs.reshape(-1), minlength=B * P).reshape(B, P)
    counts = counts.astype(np.float32)
    invc = 1.0 / np.where(counts > 0, counts, 1.0)

    # labT[b][n, t] = labels[b, t*128 + n]
    labT = labels_i.reshape(B, NCHUNK, 128).transpose(0, 2, 1).astype(np.float32)

    cst = np.empty((B, 128, CST_W), np.float32)
    cst[:, :, O_IOTA : O_IOTA + 128] = np.tile(
        np.arange(128, dtype=np.float32), (128, 1)
    )
    cst[:, :, O_LABT : O_LABT + NCHUNK] = labT
    cst[:, :, O_IDENT : O_IDENT + 128] = np.eye(128, dtype=np.float32)
    cst[:, :, O_INVC : O_INVC + 128] = invc[:, None, :]
    cst[:, :, O_W1T : O_W1T + 128] = np.asarray(W1, np.float32).T
    cst[:, :, O_W2T : O_W2T + 128] = np.asarray(W2, np.float32).T
    cst[:, :, O_W3T : O_W3T + 128] = np.asarray(W3, np.float32).T
    gb = np.stack(
        [np.asarray(a, np.float32) for a in (g1, beta1, g2, beta2, g3, beta3)],
        axis=1,
    )  # [128, 6]
    cst[:, :, O_GB : O_GB + 6] = gb

    in_maps = [
        {"feat": np.ascontiguousarray(features[b]), "cst": cst[b]} for b in range(B)
    ]
    return in_maps, counts


def kernel(features, labels, W1, b1, g1, beta1, W2, b2, g2, beta2, W3, g3, beta3,
           _debug=False):
    from concourse import bass_utils

    in_maps, counts = _host_prep(
        features, labels, W1, g1, beta1, W2, g2, beta2, W3, g3, beta3
    )
    nc = _get_compiled()
    res = bass_utils.run_bass_kernel_spmd(
        nc, in_maps, core_ids=list(range(8)), trace=_debug
    )
    x = res.results[0]["xout"]
    if _debug:
        kernel._last_results = res
    return x, counts
